# revision 5
# baseline (speedup 1.0000x reference)
"""AttentionHead kernel v5 for 8 Trainium2 NeuronCores (SPMD data-parallel).

Sharding (kv-shard): 8 cores = 4 batches x 2 KV-halves. Each core:
full query [2048, 1024] in FP8(e4m3), half key/value [1024, 1024] bf16,
inverted mask u8 [skv, sq]. Per-core DMA ~8.3 MiB.

Per-core pipeline (chunk c = 512 query columns, group g = 4 j-tiles):
  - kT/qT projections (M=64); qproj runs in fp8 (query + w_q quantized
    e4m3; only the q side is fp8 -- error ~1.3% rel, within the 2e-2
    budget); PSUM->SBUF evictions on ACT.
  - vproj: all 8 j into ONE PSUM bank, single DVE cast into vaug
    [128, 8, 65] (ones col).
  - scores per (j, c): [128, 512] transposed, pair PSUM tiles
    [128, 2, 512]; mask fused into the PSUM->SBUF eviction: ONE DVE
    tensor_tensor per pair: sb = scores * m' (u8). Masked -> exp(0)=1.
  - ACT exp per (g, c) on [128, 4, 512] SBUF bf16 (FD=2048).
  - PV: oT_c[65, 512] += vaug_j^T @ E; row 64 = denominator; oT
    eviction on DVE.
  - host: numer = oT[0:64], denom = oT[64], combine halves, divide.
"""

import numpy as np
import ml_dtypes

B = 4
S = 2048
D_MODEL = 1024
D_K = 64
N_CORES = 8

P = 128
SQ = S
SKV = S // 2
MB = D_MODEL // P
JT = SKV // P
NQC = SQ // 512
NKC = SKV // 512
NG = 2  # j-tile groups per chunk (4 j each)

_BF16 = ml_dtypes.bfloat16
_FP8 = ml_dtypes.float8_e4m3

_cached_nc = None


def _build_nc():
    import concourse.mybir as mybir
    import concourse.tile as tile
    from concourse import bacc

    bf16 = mybir.dt.bfloat16
    fp8 = mybir.dt.float8e4
    f32 = mybir.dt.float32
    u8 = mybir.dt.uint8
    Alu = mybir.AluOpType

    nc = bacc.Bacc(None, target_bir_lowering=False)

    wq_d = nc.dram_tensor("w_q", [P, MB, D_K], fp8, kind="ExternalInput")
    wkv_d = nc.dram_tensor("w_kv", [P, MB, 2 * D_K], bf16, kind="ExternalInput")
    q_d = nc.dram_tensor("q_t", [P, NQC, MB, 512], fp8, kind="ExternalInput")
    k_d = nc.dram_tensor("k_t", [P, NKC, MB, 512], bf16, kind="ExternalInput")
    v_d = nc.dram_tensor("v_t", [P, JT, MB, P], bf16, kind="ExternalInput")
    m_d = nc.dram_tensor("m_t", [P, NQC, JT, 512], u8, kind="ExternalInput")
    out_d = nc.dram_tensor("out", [D_K + 1, NQC, 512], bf16, kind="ExternalOutput")

    with tile.TileContext(nc) as tc:
        with (
            tc.tile_pool(name="const", bufs=1) as cpool,
            tc.tile_pool(name="inp", bufs=1) as ipool,
            tc.tile_pool(name="proj", bufs=1) as jpool,
            tc.tile_pool(name="ee", bufs=2) as epool,
            tc.tile_pool(name="ee3", bufs=3) as epool3,
            tc.tile_pool(name="fin", bufs=1) as fpool,
            tc.tile_pool(name="ps_pqk", bufs=2, space="PSUM") as ps_pqk,
            tc.tile_pool(name="ps_s", bufs=2, space="PSUM") as ps_s,
            tc.tile_pool(name="ps_o", bufs=2, space="PSUM") as ps_o,
        ):
            k_sb = ipool.tile([P, NKC, MB, 512], bf16, tag="k")
            q_sb = ipool.tile([P, NQC, MB, 512], fp8, tag="q")
            m_sb = ipool.tile([P, NQC, JT, 512], u8, tag="m")
            v_sb = ipool.tile([P, JT, MB, P], bf16, tag="v")
            wq_sb = cpool.tile([P, MB, D_K], fp8, tag="wq")
            wkv_sb = cpool.tile([P, MB, 2 * D_K], bf16, tag="wkv")

            def dma_k(kc):
                nc.sync.dma_start(out=k_sb[:, kc], in_=k_d[:, kc])

            def dma_q(c):
                nc.sync.dma_start(out=q_sb[:, c], in_=q_d[:, c])

            def dma_m(c):
                nc.sync.dma_start(out=m_sb[:, c], in_=m_d[:, c])

            def dma_v(g):
                nc.sync.dma_start(
                    out=v_sb[:, g * 4 : (g + 1) * 4], in_=v_d[:, g * 4 : (g + 1) * 4]
                )

            dma_k(0)
            nc.sync.dma_start(out=wkv_sb, in_=wkv_d[:])
            nc.sync.dma_start(out=wq_sb, in_=wq_d[:])
            dma_q(0)
            dma_m(0)
            dma_k(1)
            dma_q(1)
            dma_m(1)
            dma_q(2)
            dma_m(2)
            dma_v(0)
            dma_v(1)
            dma_q(3)
            dma_m(3)

            def wq(i):
                return wq_sb[:, i]

            def wk(i):
                return wkv_sb[:, i, 0:D_K]

            def wv(i):
                return wkv_sb[:, i, D_K : 2 * D_K]

            warm = cpool.tile([P, 512], bf16, tag="warm")
            nc.vector.memset(warm, 0.25)
            vaug = jpool.tile([P, JT, D_K + 1], bf16, tag="vaug")
            nc.vector.memset(vaug[:, :, D_K : D_K + 1], 1.0)

            warm_ps = ps_pqk.tile([P, 512], f32, tag="pqk", name="warm")
            for wi in range(6):
                nc.tensor.matmul(
                    warm_ps[0:D_K],
                    lhsT=warm[:, 0:D_K],
                    rhs=warm,
                    start=(wi == 0),
                    stop=(wi == 5),
                )

            qT = jpool.tile([D_K, SQ], bf16, tag="qT")
            kT = jpool.tile([D_K, SKV], bf16, tag="kT")

            def kproj(kc):
                pp = ps_pqk.tile([P, 512], f32, tag="pqk", name=f"pk{kc}")
                for i in range(MB):
                    nc.tensor.matmul(
                        pp[0:D_K],
                        lhsT=wk(i),
                        rhs=k_sb[:, kc, i],
                        start=(i == 0),
                        stop=(i == MB - 1),
                    )
                # front evictions ride the still-idle DVE, off ACT's stream
                nc.vector.tensor_copy(kT[:, kc * 512 : (kc + 1) * 512], pp[0:D_K])

            def qproj(c):
                pp = ps_pqk.tile([P, 512], f32, tag="pqk", name=f"pq{c}")
                for i in range(MB):
                    nc.tensor.matmul(
                        pp[0:D_K],
                        lhsT=wq(i),
                        rhs=q_sb[:, c, i],
                        start=(i == 0),
                        stop=(i == MB - 1),
                    )
                if c == 0:
                    nc.vector.tensor_copy(qT[:, 0:512], pp[0:D_K])
                else:
                    nc.scalar.copy(qT[:, c * 512 : (c + 1) * 512], pp[0:D_K])

            def vproj():
                pv = ps_pqk.tile([P, 512], f32, tag="pqk", name="pv")
                pvj = pv.rearrange("p (j k) -> p j k", j=JT)
                for j in range(JT):
                    for i in range(MB):
                        nc.tensor.matmul(
                            pvj[:, j],
                            lhsT=v_sb[:, j, i],
                            rhs=wv(i),
                            start=(i == 0),
                            stop=(i == MB - 1),
                        )
                nc.vector.tensor_copy(vaug[:, :, 0:D_K], pvj)

            SB = {}
            Es = {}

            def scores_g(c, g):
                """Score MMs + fused mask-eviction (DVE) for group g (pairs 2g, 2g+1)."""
                csl = slice(c * 512, (c + 1) * 512)
                for ph in range(2):
                    p = 2 * g + ph
                    sc = ps_s.tile([P, 2, 512], f32, tag="sc", name=f"sc{p}_{c}")
                    for h in range(2):
                        j = 2 * p + h
                        nc.tensor.matmul(
                            sc[:, h],
                            lhsT=kT[:, j * P : (j + 1) * P],
                            rhs=qT[:, csl],
                            start=True,
                            stop=True,
                        )
                    sb = epool.tile([P, 2, 512], bf16, tag=f"S{p}", name=f"S{p}_{c}")
                    SB[(p, c)] = sb
                    nc.vector.scalar_tensor_tensor(
                        out=sb,
                        in0=sc,
                        scalar=1.0,
                        in1=m_sb[:, c, 2 * p : 2 * p + 2],
                        op0=Alu.mult,
                        op1=Alu.mult,
                    )

            def exp_g(c, g):
                for ph in range(2):
                    p = 2 * g + ph
                    E = epool3.tile([P, 2, 512], bf16, tag=f"E{p}", name=f"E{p}_{c}")
                    nc.scalar.activation(
                        out=E,
                        in_=SB[(p, c)],
                        func=mybir.ActivationFunctionType.Exp,
                        scale=float(D_MODEL) ** -0.5,
                    )
                    Es[(p, c)] = E

            oT_sb = fpool.tile([D_K + 1, NQC, 512], bf16, tag="oT")

            def pv_chunk(c):
                po = ps_o.tile([D_K + 1, 512], f32, tag="o", name=f"o{c}")
                for j in range(JT):
                    nc.tensor.matmul(
                        po,
                        lhsT=vaug[:, j],
                        rhs=Es[(j // 2, c)][:, j % 2],
                        start=(j == 0),
                        stop=(j == JT - 1),
                    )
                nc.scalar.copy(oT_sb[:, c], po)
                nc.sync.dma_start(out=out_d[:, c], in_=oT_sb[:, c])

            # ---- emission in intended engine order ----
            kproj(0)
            qproj(0)
            scores_g(0, 0)
            exp_g(0, 0)
            kproj(1)
            scores_g(0, 1)
            exp_g(0, 1)
            qproj(1)
            scores_g(1, 0)
            exp_g(1, 0)
            scores_g(1, 1)
            exp_g(1, 1)
            qproj(2)
            scores_g(2, 0)
            exp_g(2, 0)
            scores_g(2, 1)
            exp_g(2, 1)
            vproj()
            qproj(3)
            scores_g(3, 0)
            exp_g(3, 0)
            scores_g(3, 1)
            exp_g(3, 1)
            pv_chunk(0)
            pv_chunk(1)
            pv_chunk(2)
            pv_chunk(3)

    nc.finalize()
    return nc


def _get_nc():
    global _cached_nc
    if _cached_nc is None:
        _cached_nc = _build_nc()
    return _cached_nc


def _pack_chunks(x_t, nchunks, dtype):
    s = x_t.shape[1]
    assert s == nchunks * 512
    return np.ascontiguousarray(
        x_t.astype(dtype).reshape(MB, P, nchunks, 512).transpose(1, 2, 0, 3)
    )


def _shard_inputs(query, key, value, mask, w_q, w_k, w_v):
    wq_dev = np.ascontiguousarray(
        w_q.T.astype(_FP8).reshape(MB, P, D_K).transpose(1, 0, 2)
    )
    wkv_dev = np.ascontiguousarray(
        np.concatenate(
            [
                w.T.astype(_BF16).reshape(MB, P, D_K).transpose(1, 0, 2)
                for w in (w_k, w_v)
            ],
            axis=2,
        )
    )
    in_maps = []
    for c in range(N_CORES):
        b, h = divmod(c, 2)
        s0 = h * SKV
        q_t = query[b].T
        k_t = key[b, s0 : s0 + SKV, :].T
        v_t = value[b, s0 : s0 + SKV, :].T.astype(_BF16)
        m_inv = (~mask[b, :, s0 : s0 + SKV]).astype(np.uint8).T
        m_dev = np.ascontiguousarray(
            m_inv.reshape(JT, P, NQC, 512).transpose(1, 2, 0, 3)
        )
        in_maps.append(
            {
                "w_q": wq_dev,
                "w_kv": wkv_dev,
                "q_t": _pack_chunks(q_t, NQC, _FP8),
                "k_t": _pack_chunks(k_t, NKC, _BF16),
                "v_t": np.ascontiguousarray(
                    v_t.reshape(MB, P, JT, P).transpose(1, 2, 0, 3)
                ),
                "m_t": m_dev,
            }
        )
    return in_maps


def run(inputs, trace=False):
    from concourse.bass_utils import run_bass_kernel_spmd

    nc = _get_nc()
    in_maps = _shard_inputs(**inputs)
    res = run_bass_kernel_spmd(
        nc, in_maps, core_ids=list(range(N_CORES)), trace=trace
    )

    out = np.empty((B, S, D_K), np.float32)
    for b in range(B):
        numer = np.zeros((D_K, S), np.float32)
        denom = np.zeros((S,), np.float32)
        for h in range(2):
            c = 2 * b + h
            o = np.asarray(res.results[c]["out"], np.float32).reshape(D_K + 1, S)
            numer += o[0:D_K]
            denom += o[D_K]
        out[b] = (numer / denom[None, :]).T
    return out, res


def kernel(**inputs):
    out, _ = run(inputs, trace=False)
    return out


# revision 6
# speedup vs baseline: 1.0648x; 1.0648x over previous
"""AttentionHead kernel v5 for 8 Trainium2 NeuronCores (SPMD data-parallel).

Sharding (kv-shard): 8 cores = 4 batches x 2 KV-halves. Each core:
full query [2048, 1024] in FP8(e4m3), half key/value [1024, 1024] bf16,
inverted mask u8 [skv, sq]. Per-core DMA ~8.3 MiB.

Per-core pipeline (chunk c = 512 query columns, group g = 4 j-tiles):
  - kT/qT projections (M=64); qproj runs in fp8 (query + w_q quantized
    e4m3; only the q side is fp8 -- error ~1.3% rel, within the 2e-2
    budget); PSUM->SBUF evictions on ACT.
  - vproj: all 8 j into ONE PSUM bank, single DVE cast into vaug
    [128, 8, 65] (ones col).
  - scores per (j, c): [128, 512] transposed, pair PSUM tiles
    [128, 2, 512]; mask fused into the PSUM->SBUF eviction: ONE DVE
    tensor_tensor per pair: sb = scores * m' (u8). Masked -> exp(0)=1.
  - ACT exp per (g, c) on [128, 4, 512] SBUF bf16 (FD=2048).
  - PV: oT_c[65, 512] += vaug_j^T @ E; row 64 = denominator; oT
    eviction on DVE.
  - host: numer = oT[0:64], denom = oT[64], combine halves, divide.
"""

import numpy as np
import ml_dtypes

B = 4
S = 2048
D_MODEL = 1024
D_K = 64
N_CORES = 8

P = 128
SQ = S
SKV = S // 2
MB = D_MODEL // P
JT = SKV // P
NQC = SQ // 512
NKC = SKV // 512
NG = 2  # j-tile groups per chunk (4 j each)

_BF16 = ml_dtypes.bfloat16
_FP8 = ml_dtypes.float8_e4m3

_cached_nc = None


def _build_nc():
    import concourse.mybir as mybir
    import concourse.tile as tile
    from concourse import bacc

    bf16 = mybir.dt.bfloat16
    fp8 = mybir.dt.float8e4
    f32 = mybir.dt.float32
    u8 = mybir.dt.uint8
    Alu = mybir.AluOpType

    nc = bacc.Bacc(None, target_bir_lowering=False)

    wq_d = nc.dram_tensor("w_q", [P, MB, D_K], fp8, kind="ExternalInput")
    wkv_d = nc.dram_tensor("w_kv", [P, MB, 2 * D_K], bf16, kind="ExternalInput")
    q_d = nc.dram_tensor("q_t", [P, NQC, MB, 512], fp8, kind="ExternalInput")
    k_d = nc.dram_tensor("k_t", [P, NKC, MB, 512], bf16, kind="ExternalInput")
    v_d = nc.dram_tensor("v_t", [P, JT, MB, P], bf16, kind="ExternalInput")
    m_d = nc.dram_tensor("m_t", [P, NQC, JT, 512], u8, kind="ExternalInput")
    out_d = nc.dram_tensor("out", [D_K + 1, NQC, 512], bf16, kind="ExternalOutput")

    with tile.TileContext(nc) as tc:
        with (
            tc.tile_pool(name="const", bufs=1) as cpool,
            tc.tile_pool(name="inp", bufs=1) as ipool,
            tc.tile_pool(name="proj", bufs=1) as jpool,
            tc.tile_pool(name="ee", bufs=2) as epool,
            tc.tile_pool(name="ee3", bufs=3) as epool3,
            tc.tile_pool(name="fin", bufs=1) as fpool,
            tc.tile_pool(name="ps_pqk", bufs=2, space="PSUM") as ps_pqk,
            tc.tile_pool(name="ps_s", bufs=2, space="PSUM") as ps_s,
            tc.tile_pool(name="ps_o", bufs=2, space="PSUM") as ps_o,
        ):
            k_sb = ipool.tile([P, NKC, MB, 512], bf16, tag="k")
            q_sb = ipool.tile([P, NQC, MB, 512], fp8, tag="q")
            m_sb = ipool.tile([P, NQC, JT, 512], u8, tag="m")
            v_sb = ipool.tile([P, JT, MB, P], bf16, tag="v")
            wq_sb = cpool.tile([P, MB, D_K], fp8, tag="wq")
            wkv_sb = cpool.tile([P, MB, 2 * D_K], bf16, tag="wkv")

            def dma_k(kc):
                nc.sync.dma_start(out=k_sb[:, kc], in_=k_d[:, kc])

            def dma_q(c):
                nc.sync.dma_start(out=q_sb[:, c], in_=q_d[:, c])

            def dma_m(c):
                nc.sync.dma_start(out=m_sb[:, c], in_=m_d[:, c])

            def dma_v(g):
                nc.sync.dma_start(
                    out=v_sb[:, g * 4 : (g + 1) * 4], in_=v_d[:, g * 4 : (g + 1) * 4]
                )

            dma_k(0)
            nc.sync.dma_start(out=wkv_sb, in_=wkv_d[:])
            nc.sync.dma_start(out=wq_sb, in_=wq_d[:])
            dma_q(0)
            dma_m(0)
            dma_k(1)
            dma_q(1)
            dma_m(1)
            dma_q(2)
            dma_m(2)
            dma_v(0)
            dma_v(1)
            dma_q(3)
            dma_m(3)

            def wq(i):
                return wq_sb[:, i]

            def wk(i):
                return wkv_sb[:, i, 0:D_K]

            def wv(i):
                return wkv_sb[:, i, D_K : 2 * D_K]

            warm = cpool.tile([P, 512], bf16, tag="warm")
            nc.vector.memset(warm, 0.25)
            vaug = jpool.tile([P, JT, D_K + 1], bf16, tag="vaug")
            nc.vector.memset(vaug[:, :, D_K : D_K + 1], 1.0)

            warm_ps = ps_pqk.tile([P, 512], f32, tag="pqk", name="warm")
            for wi in range(6):
                nc.tensor.matmul(
                    warm_ps[0:D_K],
                    lhsT=warm[:, 0:D_K],
                    rhs=warm,
                    start=(wi == 0),
                    stop=(wi == 5),
                )

            qT = jpool.tile([D_K, SQ], bf16, tag="qT")
            kT = jpool.tile([D_K, SKV], bf16, tag="kT")

            def kproj(kc):
                pp = ps_pqk.tile([P, 512], f32, tag="pqk", name=f"pk{kc}")
                for i in range(MB):
                    nc.tensor.matmul(
                        pp[0:D_K],
                        lhsT=wk(i),
                        rhs=k_sb[:, kc, i],
                        start=(i == 0),
                        stop=(i == MB - 1),
                    )
                # front evictions ride the still-idle DVE, off ACT's stream
                nc.vector.tensor_copy(kT[:, kc * 512 : (kc + 1) * 512], pp[0:D_K])

            def qproj(c):
                pp = ps_pqk.tile([P, 512], f32, tag="pqk", name=f"pq{c}")
                for i in range(MB):
                    nc.tensor.matmul(
                        pp[0:D_K],
                        lhsT=wq(i),
                        rhs=q_sb[:, c, i],
                        start=(i == 0),
                        stop=(i == MB - 1),
                    )
                if c == 0:
                    nc.vector.tensor_copy(qT[:, 0:512], pp[0:D_K])
                else:
                    nc.scalar.copy(qT[:, c * 512 : (c + 1) * 512], pp[0:D_K])

            def vproj():
                pv = ps_pqk.tile([P, 512], f32, tag="pqk", name="pv")
                pvj = pv.rearrange("p (j k) -> p j k", j=JT)
                for j in range(JT):
                    for i in range(MB):
                        nc.tensor.matmul(
                            pvj[:, j],
                            lhsT=v_sb[:, j, i],
                            rhs=wv(i),
                            start=(i == 0),
                            stop=(i == MB - 1),
                        )
                nc.vector.tensor_copy(vaug[:, :, 0:D_K], pvj)

            SB = {}
            Es = {}

            def scores_g(c, g):
                """Score MMs + fused mask-eviction (DVE) for group g (pairs 2g, 2g+1)."""
                csl = slice(c * 512, (c + 1) * 512)
                for ph in range(2):
                    p = 2 * g + ph
                    sc = ps_s.tile([P, 2, 512], f32, tag="sc", name=f"sc{p}_{c}")
                    for h in range(2):
                        j = 2 * p + h
                        nc.tensor.matmul(
                            sc[:, h],
                            lhsT=kT[:, j * P : (j + 1) * P],
                            rhs=qT[:, csl],
                            start=True,
                            stop=True,
                        )
                    sb = epool.tile([P, 2, 512], bf16, tag=f"S{p}", name=f"S{p}_{c}")
                    SB[(p, c)] = sb
                    nc.vector.tensor_tensor(
                        out=sb,
                        in0=sc,
                        in1=m_sb[:, c, 2 * p : 2 * p + 2],
                        op=Alu.mult,
                    )

            def exp_g(c, g):
                for ph in range(2):
                    p = 2 * g + ph
                    E = epool3.tile([P, 2, 512], bf16, tag=f"E{p}", name=f"E{p}_{c}")
                    nc.scalar.activation(
                        out=E,
                        in_=SB[(p, c)],
                        func=mybir.ActivationFunctionType.Exp,
                        scale=float(D_MODEL) ** -0.5,
                    )
                    Es[(p, c)] = E

            oT_sb = fpool.tile([D_K + 1, NQC, 512], bf16, tag="oT")

            def pv_chunk(c):
                po = ps_o.tile([D_K + 1, 512], f32, tag="o", name=f"o{c}")
                for j in range(JT):
                    nc.tensor.matmul(
                        po,
                        lhsT=vaug[:, j],
                        rhs=Es[(j // 2, c)][:, j % 2],
                        start=(j == 0),
                        stop=(j == JT - 1),
                    )
                nc.scalar.copy(oT_sb[:, c], po)
                nc.sync.dma_start(out=out_d[:, c], in_=oT_sb[:, c])

            # ---- emission in intended engine order ----
            kproj(0)
            qproj(0)
            scores_g(0, 0)
            exp_g(0, 0)
            kproj(1)
            scores_g(0, 1)
            exp_g(0, 1)
            qproj(1)
            scores_g(1, 0)
            exp_g(1, 0)
            scores_g(1, 1)
            exp_g(1, 1)
            qproj(2)
            scores_g(2, 0)
            exp_g(2, 0)
            scores_g(2, 1)
            exp_g(2, 1)
            vproj()
            qproj(3)
            scores_g(3, 0)
            exp_g(3, 0)
            scores_g(3, 1)
            exp_g(3, 1)
            pv_chunk(0)
            pv_chunk(1)
            pv_chunk(2)
            pv_chunk(3)

    nc.finalize()
    return nc


def _get_nc():
    global _cached_nc
    if _cached_nc is None:
        _cached_nc = _build_nc()
    return _cached_nc


def _pack_chunks(x_t, nchunks, dtype):
    s = x_t.shape[1]
    assert s == nchunks * 512
    return np.ascontiguousarray(
        x_t.astype(dtype).reshape(MB, P, nchunks, 512).transpose(1, 2, 0, 3)
    )


def _shard_inputs(query, key, value, mask, w_q, w_k, w_v):
    wq_dev = np.ascontiguousarray(
        w_q.T.astype(_FP8).reshape(MB, P, D_K).transpose(1, 0, 2)
    )
    wkv_dev = np.ascontiguousarray(
        np.concatenate(
            [
                w.T.astype(_BF16).reshape(MB, P, D_K).transpose(1, 0, 2)
                for w in (w_k, w_v)
            ],
            axis=2,
        )
    )
    in_maps = []
    for c in range(N_CORES):
        b, h = divmod(c, 2)
        s0 = h * SKV
        q_t = query[b].T
        k_t = key[b, s0 : s0 + SKV, :].T
        v_t = value[b, s0 : s0 + SKV, :].T.astype(_BF16)
        m_inv = (~mask[b, :, s0 : s0 + SKV]).astype(np.uint8).T
        m_dev = np.ascontiguousarray(
            m_inv.reshape(JT, P, NQC, 512).transpose(1, 2, 0, 3)
        )
        in_maps.append(
            {
                "w_q": wq_dev,
                "w_kv": wkv_dev,
                "q_t": _pack_chunks(q_t, NQC, _FP8),
                "k_t": _pack_chunks(k_t, NKC, _BF16),
                "v_t": np.ascontiguousarray(
                    v_t.reshape(MB, P, JT, P).transpose(1, 2, 0, 3)
                ),
                "m_t": m_dev,
            }
        )
    return in_maps


def run(inputs, trace=False):
    from concourse.bass_utils import run_bass_kernel_spmd

    nc = _get_nc()
    in_maps = _shard_inputs(**inputs)
    res = run_bass_kernel_spmd(
        nc, in_maps, core_ids=list(range(N_CORES)), trace=trace
    )

    out = np.empty((B, S, D_K), np.float32)
    for b in range(B):
        numer = np.zeros((D_K, S), np.float32)
        denom = np.zeros((S,), np.float32)
        for h in range(2):
            c = 2 * b + h
            o = np.asarray(res.results[c]["out"], np.float32).reshape(D_K + 1, S)
            numer += o[0:D_K]
            denom += o[D_K]
        out[b] = (numer / denom[None, :]).T
    return out, res


def kernel(**inputs):
    out, _ = run(inputs, trace=False)
    return out


# revision 7
# speedup vs baseline: 1.0816x; 1.0158x over previous
"""AttentionHead kernel v5 for 8 Trainium2 NeuronCores (SPMD data-parallel).

Sharding (kv-shard): 8 cores = 4 batches x 2 KV-halves. Each core:
full query [2048, 1024] in FP8(e4m3), half key/value [1024, 1024] bf16,
inverted mask u8 [skv, sq]. Per-core DMA ~8.3 MiB.

Per-core pipeline (chunk c = 512 query columns, group g = 4 j-tiles):
  - kT/qT projections (M=64); qproj runs in fp8 (query + w_q quantized
    e4m3; only the q side is fp8 -- error ~1.3% rel, within the 2e-2
    budget); PSUM->SBUF evictions on ACT.
  - vproj: all 8 j into ONE PSUM bank, single DVE cast into vaug
    [128, 8, 65] (ones col).
  - scores per (j, c): [128, 512] transposed, pair PSUM tiles
    [128, 2, 512]; mask fused into the PSUM->SBUF eviction: ONE DVE
    tensor_tensor per pair: sb = scores * m' (u8). Masked -> exp(0)=1.
  - ACT exp per (g, c) on [128, 4, 512] SBUF bf16 (FD=2048).
  - PV: oT_c[65, 512] += vaug_j^T @ E; row 64 = denominator; oT
    eviction on DVE.
  - host: numer = oT[0:64], denom = oT[64], combine halves, divide.
"""

import numpy as np
import ml_dtypes

B = 4
S = 2048
D_MODEL = 1024
D_K = 64
N_CORES = 8

P = 128
SQ = S
SKV = S // 2
MB = D_MODEL // P
JT = SKV // P
NQC = SQ // 512
NKC = SKV // 512
NG = 2  # j-tile groups per chunk (4 j each)

_BF16 = ml_dtypes.bfloat16
_FP8 = ml_dtypes.float8_e4m3

_cached_nc = None


def _build_nc():
    import concourse.mybir as mybir
    import concourse.tile as tile
    from concourse import bacc

    bf16 = mybir.dt.bfloat16
    fp8 = mybir.dt.float8e4
    f32 = mybir.dt.float32
    u8 = mybir.dt.uint8
    Alu = mybir.AluOpType

    nc = bacc.Bacc(None, target_bir_lowering=False)

    wq_d = nc.dram_tensor("w_q", [P, MB, D_K], fp8, kind="ExternalInput")
    wkv_d = nc.dram_tensor("w_kv", [P, MB, 2 * D_K], bf16, kind="ExternalInput")
    q_d = nc.dram_tensor("q_t", [P, NQC, MB, 512], fp8, kind="ExternalInput")
    k_d = nc.dram_tensor("k_t", [P, NKC, MB, 512], bf16, kind="ExternalInput")
    v_d = nc.dram_tensor("v_t", [P, JT, MB, P], bf16, kind="ExternalInput")
    m_d = nc.dram_tensor("m_t", [P, NQC, JT, 512], u8, kind="ExternalInput")
    out_d = nc.dram_tensor("out", [D_K + 1, NQC, 512], bf16, kind="ExternalOutput")

    with tile.TileContext(nc) as tc:
        with (
            tc.tile_pool(name="const", bufs=1) as cpool,
            tc.tile_pool(name="inp", bufs=1) as ipool,
            tc.tile_pool(name="proj", bufs=1) as jpool,
            tc.tile_pool(name="ee", bufs=2) as epool,
            tc.tile_pool(name="ee3", bufs=3) as epool3,
            tc.tile_pool(name="fin", bufs=1) as fpool,
            tc.tile_pool(name="ps_pqk", bufs=2, space="PSUM") as ps_pqk,
            tc.tile_pool(name="ps_s", bufs=2, space="PSUM") as ps_s,
            tc.tile_pool(name="ps_o", bufs=2, space="PSUM") as ps_o,
        ):
            k_sb = ipool.tile([P, NKC, MB, 512], bf16, tag="k")
            q_sb = ipool.tile([P, NQC, MB, 512], fp8, tag="q")
            m_sb = ipool.tile([P, NQC, JT, 512], u8, tag="m")
            v_sb = ipool.tile([P, JT, MB, P], bf16, tag="v")
            wq_sb = cpool.tile([P, MB, D_K], fp8, tag="wq")
            wkv_sb = cpool.tile([P, MB, 2 * D_K], bf16, tag="wkv")

            def dma_k(kc):
                nc.sync.dma_start(out=k_sb[:, kc], in_=k_d[:, kc])

            def dma_q(c):
                nc.sync.dma_start(out=q_sb[:, c], in_=q_d[:, c])

            def dma_m(c):
                nc.sync.dma_start(out=m_sb[:, c], in_=m_d[:, c])

            def dma_v(g):
                nc.sync.dma_start(
                    out=v_sb[:, g * 4 : (g + 1) * 4], in_=v_d[:, g * 4 : (g + 1) * 4]
                )

            dma_k(0)
            nc.sync.dma_start(out=wkv_sb, in_=wkv_d[:])
            nc.sync.dma_start(out=wq_sb, in_=wq_d[:])
            dma_q(0)
            dma_m(0)
            dma_k(1)
            dma_q(1)
            dma_m(1)
            dma_q(2)
            dma_m(2)
            dma_v(0)
            dma_v(1)
            dma_q(3)
            dma_m(3)

            def wq(i):
                return wq_sb[:, i]

            def wk(i):
                return wkv_sb[:, i, 0:D_K]

            def wv(i):
                return wkv_sb[:, i, D_K : 2 * D_K]

            vaug = jpool.tile([P, JT, D_K + 1], bf16, tag="vaug")
            nc.vector.memset(vaug[:, :, D_K : D_K + 1], 1.0)

            qT = jpool.tile([D_K, SQ], bf16, tag="qT")
            kT = jpool.tile([D_K, SKV], bf16, tag="kT")

            def kproj(kc):
                pp = ps_pqk.tile([P, 512], f32, tag="pqk", name=f"pk{kc}")
                for i in range(MB):
                    nc.tensor.matmul(
                        pp[0:D_K],
                        lhsT=wk(i),
                        rhs=k_sb[:, kc, i],
                        start=(i == 0),
                        stop=(i == MB - 1),
                    )
                # front evictions ride the still-idle DVE, off ACT's stream
                nc.vector.tensor_copy(kT[:, kc * 512 : (kc + 1) * 512], pp[0:D_K])

            def qproj(c):
                pp = ps_pqk.tile([P, 512], f32, tag="pqk", name=f"pq{c}")
                for i in range(MB):
                    nc.tensor.matmul(
                        pp[0:D_K],
                        lhsT=wq(i),
                        rhs=q_sb[:, c, i],
                        start=(i == 0),
                        stop=(i == MB - 1),
                    )
                if c == 0:
                    nc.vector.tensor_copy(qT[:, 0:512], pp[0:D_K])
                else:
                    nc.scalar.copy(qT[:, c * 512 : (c + 1) * 512], pp[0:D_K])

            def vproj():
                pv = ps_pqk.tile([P, 512], f32, tag="pqk", name="pv")
                pvj = pv.rearrange("p (j k) -> p j k", j=JT)
                for j in range(JT):
                    for i in range(MB):
                        nc.tensor.matmul(
                            pvj[:, j],
                            lhsT=v_sb[:, j, i],
                            rhs=wv(i),
                            start=(i == 0),
                            stop=(i == MB - 1),
                        )
                nc.vector.tensor_copy(vaug[:, :, 0:D_K], pvj)

            SB = {}
            Es = {}

            def scores_g(c, g):
                """Score MMs + fused mask-eviction (DVE) for group g (pairs 2g, 2g+1)."""
                csl = slice(c * 512, (c + 1) * 512)
                for ph in range(2):
                    p = 2 * g + ph
                    sc = ps_s.tile([P, 2, 512], f32, tag="sc", name=f"sc{p}_{c}")
                    for h in range(2):
                        j = 2 * p + h
                        nc.tensor.matmul(
                            sc[:, h],
                            lhsT=kT[:, j * P : (j + 1) * P],
                            rhs=qT[:, csl],
                            start=True,
                            stop=True,
                        )
                    sb = epool.tile([P, 2, 512], bf16, tag=f"S{p}", name=f"S{p}_{c}")
                    SB[(p, c)] = sb
                    nc.vector.tensor_tensor(
                        out=sb,
                        in0=sc,
                        in1=m_sb[:, c, 2 * p : 2 * p + 2],
                        op=Alu.mult,
                    )

            def exp_g(c, g):
                for ph in range(2):
                    p = 2 * g + ph
                    E = epool3.tile([P, 2, 512], bf16, tag=f"E{p}", name=f"E{p}_{c}")
                    nc.scalar.activation(
                        out=E,
                        in_=SB[(p, c)],
                        func=mybir.ActivationFunctionType.Exp,
                        scale=float(D_MODEL) ** -0.5,
                    )
                    Es[(p, c)] = E

            oT_sb = fpool.tile([D_K + 1, NQC, 512], bf16, tag="oT")

            def pv_chunk(c):
                po = ps_o.tile([D_K + 1, 512], f32, tag="o", name=f"o{c}")
                for j in range(JT):
                    nc.tensor.matmul(
                        po,
                        lhsT=vaug[:, j],
                        rhs=Es[(j // 2, c)][:, j % 2],
                        start=(j == 0),
                        stop=(j == JT - 1),
                    )
                nc.scalar.copy(oT_sb[:, c], po)
                nc.sync.dma_start(out=out_d[:, c], in_=oT_sb[:, c])

            # ---- emission in intended engine order ----
            kproj(0)
            qproj(0)
            scores_g(0, 0)
            exp_g(0, 0)
            kproj(1)
            scores_g(0, 1)
            exp_g(0, 1)
            qproj(1)
            scores_g(1, 0)
            exp_g(1, 0)
            scores_g(1, 1)
            exp_g(1, 1)
            qproj(2)
            scores_g(2, 0)
            exp_g(2, 0)
            scores_g(2, 1)
            exp_g(2, 1)
            vproj()
            qproj(3)
            scores_g(3, 0)
            exp_g(3, 0)
            scores_g(3, 1)
            exp_g(3, 1)
            pv_chunk(0)
            pv_chunk(1)
            pv_chunk(2)
            pv_chunk(3)

    nc.finalize()
    return nc


def _get_nc():
    global _cached_nc
    if _cached_nc is None:
        _cached_nc = _build_nc()
    return _cached_nc


def _pack_chunks(x_t, nchunks, dtype):
    s = x_t.shape[1]
    assert s == nchunks * 512
    return np.ascontiguousarray(
        x_t.astype(dtype).reshape(MB, P, nchunks, 512).transpose(1, 2, 0, 3)
    )


def _shard_inputs(query, key, value, mask, w_q, w_k, w_v):
    wq_dev = np.ascontiguousarray(
        w_q.T.astype(_FP8).reshape(MB, P, D_K).transpose(1, 0, 2)
    )
    wkv_dev = np.ascontiguousarray(
        np.concatenate(
            [
                w.T.astype(_BF16).reshape(MB, P, D_K).transpose(1, 0, 2)
                for w in (w_k, w_v)
            ],
            axis=2,
        )
    )
    in_maps = []
    for c in range(N_CORES):
        b, h = divmod(c, 2)
        s0 = h * SKV
        q_t = query[b].T
        k_t = key[b, s0 : s0 + SKV, :].T
        v_t = value[b, s0 : s0 + SKV, :].T.astype(_BF16)
        m_inv = (~mask[b, :, s0 : s0 + SKV]).astype(np.uint8).T
        m_dev = np.ascontiguousarray(
            m_inv.reshape(JT, P, NQC, 512).transpose(1, 2, 0, 3)
        )
        in_maps.append(
            {
                "w_q": wq_dev,
                "w_kv": wkv_dev,
                "q_t": _pack_chunks(q_t, NQC, _FP8),
                "k_t": _pack_chunks(k_t, NKC, _BF16),
                "v_t": np.ascontiguousarray(
                    v_t.reshape(MB, P, JT, P).transpose(1, 2, 0, 3)
                ),
                "m_t": m_dev,
            }
        )
    return in_maps


def run(inputs, trace=False):
    from concourse.bass_utils import run_bass_kernel_spmd

    nc = _get_nc()
    in_maps = _shard_inputs(**inputs)
    res = run_bass_kernel_spmd(
        nc, in_maps, core_ids=list(range(N_CORES)), trace=trace
    )

    out = np.empty((B, S, D_K), np.float32)
    for b in range(B):
        numer = np.zeros((D_K, S), np.float32)
        denom = np.zeros((S,), np.float32)
        for h in range(2):
            c = 2 * b + h
            o = np.asarray(res.results[c]["out"], np.float32).reshape(D_K + 1, S)
            numer += o[0:D_K]
            denom += o[D_K]
        out[b] = (numer / denom[None, :]).T
    return out, res


def kernel(**inputs):
    out, _ = run(inputs, trace=False)
    return out
